# revision 9
# baseline (speedup 1.0000x reference)
"""GraphMAE-style GNN forward (3x GCNConv + BN + PReLU + SCE loss) on 8 TRN2
NeuronCores via Bass/Tile — v3, optimized for wall-clock (host+compile+exec).

Dataflow (per core, nodes sharded 6272/core, feature-major activations):
  table0 = dinv * x-masked (node-major bf16, built straight from x_shard)
  AllGather -> agg (indirect row gathers + one-hot segment-sum matmuls with
  self-loops folded into the edge list as ordinary edges) -> u1 [128,SHARD]
  -> v1 = W1^T u1 (feature-major, 512-wide rhs groups) -> BN(AllReduce)+PReLU
  -> h1.  L2/L3: table = dinv*(h W) node-major via lhsT=h matmuls (no
  transposes), AllGather, aggregate (128 feats), BN+PReLU.  Loss: per-core
  masked-row cosine partial sums + AllReduce.

Conv biases are skipped: training-mode BatchNorm cancels them exactly.
"""
import sys

sys.path.insert(0, "/opt/trn_rl_repo")
import numpy as np
import ml_dtypes

import concourse.bass as bass
import concourse.mybir as mybir
import concourse.tile as tile
from bass_rust import SyncInfo
from concourse.bass import IndirectOffsetOnAxis
from concourse.bass_utils import run_bass_kernel_spmd
from concourse.tile_rust import add_dep_helper
from concourse.vector_clock import ScopedClock

BF = ml_dtypes.bfloat16
F32 = mybir.dt.float32
BF16 = mybir.dt.bfloat16
I32 = mybir.dt.int32
OP = mybir.AluOpType

N, E, IN, HID, OUT = 50000, 800000, 128, 256, 128
NC, P = 8, 128
SHARD, T = 6272, 49
NPAD = NC * SHARD
NMASK = 25000
EPS = 1e-5
RG = [list(range(NC))]
B = 6          # chunks per batched one-hot build
GW = 512       # node-group width for the feature-major W matmuls

# ---------------------------------------------------------------------------
# Walrus on this toolchain rejects >1 semaphore wait per instruction
# ("Too many sync wait commands"). Post-process lowered blocks: hoist excess
# waits onto injected same-engine NoOps (program order => equivalent).
_uid = [0]


def _split_bb_waits(nc):
    for f in nc.m.functions:
        for bb in f.blocks:
            insts = list(bb.instructions)
            out = []
            changed = False
            for inst in insts:
                si = inst.sync_info
                waits = list(si.on_wait) if si is not None and si.on_wait else []
                if len(waits) > 1:
                    changed = True
                    rest = waits[:-1]
                    inst.sync_info.on_wait = waits[-1:]
                    while rest:
                        _uid[0] += 1
                        nop = mybir.InstNoOp(
                            name=f"I-waitsplit-{_uid[0]}", ins=[], outs=[])
                        nop.engine = inst.engine
                        nop.sync_info = SyncInfo(
                            on_wait=rest[:1], on_update=[])
                        rest = rest[1:]
                        out.append(nop)
                out.append(inst)
            if changed:
                bb.instructions = out


class TileContextSplitDrain(tile.TileContext):
    def _drain_and_barrier(self, tick_clock, wait_clock):
        nc = self.nc
        probe = nc.sync.nop(nofuse=True)
        wait_clock.add_sem_waits(
            probe.ins, ScopedClock({None: tick_clock.global_clock}))
        nc.sync.drain()
        nc.all_engine_barrier()
        assert self.sems is not None
        popped = nc._tile_sem_poison_stack.pop()
        assert popped is self._sem_poison
        nc.clear_and_free_semaphores(list(self.sems.allocated().values()))
        nc.all_engine_barrier()
        _split_bb_waits(nc)


# ---------------------------------------------------------------------------
def _prep(edge_index, mask_nodes):
    src = edge_index[0].astype(np.int64)
    dst = edge_index[1].astype(np.int64)
    deg = np.bincount(dst, minlength=N).astype(np.float64) + 1.0
    dinv = (1.0 / np.sqrt(deg)).astype(np.float32)
    dinv_pad = np.zeros(NPAD, np.float32)
    dinv_pad[:N] = dinv

    # fold self-loops in as ordinary edges (table rows carry dinv_src, the
    # post-aggregation dinv_dst factor supplies the other dinv)
    loops = np.arange(N, dtype=np.int64)
    src = np.concatenate([src, loops])
    dst = np.concatenate([dst, loops])

    mask_set = np.zeros(N, bool)
    mask_set[mask_nodes] = True

    pertile = []
    kc = 0
    core_of = dst // SHARD
    for c in range(NC):
        sel = core_of == c
        es, ed = src[sel], dst[sel] - c * SHARD
        tl = ed // P
        order = np.argsort(tl, kind="stable")
        es, ed, tl = es[order], ed[order], tl[order]
        tiles = []
        for t in range(T):
            m = tl == t
            tiles.append((es[m], (ed[m] % P).astype(np.float32)))
            kc = max(kc, (int(m.sum()) + P - 1) // P)
        pertile.append(tiles)
    kc = ((kc + B - 1) // B) * B  # pad to batch multiple

    srcs, dstl = [], []
    for c in range(NC):
        sc = np.zeros((T * kc, P), np.int32)
        dc = np.full((T * kc, P), 255.0, np.float32)
        for t in range(T):
            es, dl = pertile[c][t]
            n = len(es)
            sc[t * kc:(t + 1) * kc].reshape(-1)[:n] = es
            dc[t * kc:(t + 1) * kc].reshape(-1)[:n] = dl
        srcs.append(np.ascontiguousarray(sc.T))
        dstl.append(np.ascontiguousarray(dc.T.astype(BF)))

    mlocal, mvalid, mkeep = [], [], []
    locs = []
    mc = 0
    for c in range(NC):
        rows = np.arange(c * SHARD, (c + 1) * SHARD)
        valid = rows < N
        mm = np.zeros(SHARD, bool)
        mm[:valid.sum()] = mask_set[rows[valid]]
        loc = np.where(mm)[0]
        mc = max(mc, (len(loc) + P - 1) // P)
        locs.append(loc)
        keep = np.ones(SHARD, np.float32)
        keep[mm] = 0.0
        mkeep.append(keep)
    for c in range(NC):
        loc = locs[c]
        n = len(loc)
        lo = np.zeros((mc, P), np.int32)
        va = np.zeros((mc, P), np.float32)
        lo.reshape(-1)[:n] = loc
        va.reshape(-1)[:n] = 1.0
        mlocal.append(np.ascontiguousarray(lo.T))
        mvalid.append(np.ascontiguousarray(va.T))
    return dinv_pad, srcs, dstl, kc, mlocal, mvalid, mkeep, mc


def build_nc(kc, mc):
    nc = bass.Bass(num_devices=NC, target_bir_lowering=False)
    D = {}

    def inp(name, shape, dt):
        D[name] = nc.dram_tensor(name, shape, dt, kind="ExternalInput")
        return D[name]

    x_shard = inp("x_shard", [SHARD, IN], F32)
    inp("src_rows", [P, T * kc], I32)
    inp("dst_local", [P, T * kc], BF16)
    inp("dinv_row", [1, SHARD], F32)
    inp("cols", [P, 4 * T], F32)   # l1x | l1tok | d | l3 blocks of T
    inp("token_rep", [P, IN], F32)
    inp("iota_b", [P, B * P], BF16)
    inp("ident_bf", [P, P], BF16)
    inp("ones_col", [P, 1], F32)
    inp("w1", [IN, HID], BF16)
    inp("w2a", [P, OUT], BF16)
    inp("w2b", [P, OUT], BF16)
    inp("wd", [OUT, IN], BF16)
    inp("gb", [P, 8], F32)
    inp("a_rep", [P, 3], F32)
    inp("mrow_l", [P, mc], I32)
    inp("mval", [P, mc], F32)
    out_t = nc.dram_tensor("loss", [1, 1], F32, kind="ExternalOutput")

    NB = kc // B           # one-hot batches per dst tile
    NG = (SHARD + GW - 1) // GW   # node groups for L1 W matmul

    with TileContextSplitDrain(nc) as tc:
        with (
            tc.tile_pool(name="const", bufs=1) as cpool,
            tc.tile_pool(name="hbuf", bufs=1) as hpool,
            tc.tile_pool(name="work", bufs=2) as wk,
            tc.tile_pool(name="gpool", bufs=4) as gp,
            tc.tile_pool(name="psmm", bufs=2, space="PSUM") as psmm,
            tc.tile_pool(name="psv", bufs=2, space="PSUM") as psv,
            tc.tile_pool(name="pstr", bufs=1, space="PSUM") as pstr,
            tc.tile_pool(name="psagg", bufs=2, space="PSUM") as psagg,
            tc.tile_pool(name="dram", bufs=1, space="DRAM") as dpool,
        ):
            def load(tname):
                h = D[tname]
                t_ = cpool.tile(list(h.shape), h.dtype, tag=tname, name=tname)
                nc.sync.dma_start(t_[:], h[:])
                return t_

            src_s = load("src_rows")
            dstl_s = load("dst_local")
            cols_s = load("cols")
            tokr_s = load("token_rep")
            iota_s = load("iota_b")
            idbf_s = load("ident_bf")
            ones_s = load("ones_col")
            w1_s = load("w1")
            w2a_s = load("w2a")
            w2b_s = load("w2b")
            wd_s = load("wd")
            gb_s = load("gb")
            a_s = load("a_rep")
            mrl_s = load("mrow_l")
            mv_s = load("mval")
            dinv_s = cpool.tile([P, SHARD], F32, tag="dinv_rep",
                                name="dinv_rep")
            nc.sync.dma_start(dinv_s[:],
                              D["dinv_row"][0:1, :].to_broadcast([P, SHARD]))

            def ccol(blk, b):
                return cols_s[:, blk * T + b:blk * T + b + 1]

            table1 = dpool.tile([NPAD, IN], BF16, addr_space="Shared",
                                name="table1")
            table2 = dpool.tile([NPAD, OUT], BF16, addr_space="Shared",
                                name="table2")
            table3 = dpool.tile([NPAD, IN], BF16, addr_space="Shared",
                                name="table3")
            agin1 = dpool.tile([SHARD, IN], BF16, name="agin1")
            agin2 = dpool.tile([SHARD, OUT], BF16, name="agin2")
            agin3 = dpool.tile([SHARD, IN], BF16, name="agin3")
            loss_in = dpool.tile([1, 16], F32, name="loss_in")
            loss_out = dpool.tile([1, 16], F32, addr_space="Shared",
                                  name="loss_out")
            rex_dram = dpool.tile([SHARD, IN], BF16, name="rex_dram")

            # ---- table0: dinv * masked-x, node-major bf16 ----------------
            for b in range(T):
                xt = wk.tile([P, IN], F32, tag="xt", name="xt")
                nc.sync.dma_start(xt[:], x_shard[b * P:(b + 1) * P, :])
                tb = wk.tile([P, IN], BF16, tag="tb0", name="tb0")
                tk = wk.tile([P, IN], F32, tag="tk0", name="tk0")
                nc.vector.tensor_scalar(out=tk[:], in0=tokr_s[:],
                                        scalar1=ccol(1, b), scalar2=None,
                                        op0=OP.mult)
                nc.vector.tensor_scalar(out=xt[:], in0=xt[:],
                                        scalar1=ccol(0, b), scalar2=None,
                                        op0=OP.mult)
                nc.vector.tensor_tensor(out=tb[:], in0=xt[:], in1=tk[:],
                                        op=OP.add)
                nc.sync.dma_start(agin1[b * P:(b + 1) * P, :], tb[:])

            def allgather(agin, table):
                cc = nc.gpsimd.collective_compute(
                    "AllGather", OP.bypass, replica_groups=RG,
                    ins=[agin[:].opt()], outs=[table[:].opt()])
                fence = nc.gpsimd.memset(
                    wk.tile([1, 1], F32, tag="fence", name="fence")[:], 0.0)
                add_dep_helper(fence.ins, cc.ins, True, "fence cc")

            def aggregate(table, f_out, u_tile):
                """u = dinv_dst * sum_edges table[src], feature-major bf16."""
                for t in range(T):
                    pa = psagg.tile([P, P], F32, tag="pa", name="pa")
                    for bb in range(NB):
                        j0 = t * kc + bb * B
                        G = gp.tile([P, B * f_out], BF16, tag="G", name="G")
                        for q in range(B):
                            nc.gpsimd.indirect_dma_start(
                                out=G[:, q * f_out:(q + 1) * f_out],
                                out_offset=None, in_=table[:],
                                in_offset=IndirectOffsetOnAxis(
                                    ap=src_s[:, j0 + q:j0 + q + 1], axis=0))
                        S = gp.tile([P, B * P], BF16, tag="S", name="S")
                        nc.vector.tensor_tensor(
                            out=S[:].rearrange("p (m q) -> p m q", m=B),
                            in0=dstl_s[:, j0:j0 + B].unsqueeze(2)
                                .to_broadcast([P, B, P]),
                            in1=iota_s[:].rearrange("p (m q) -> p m q", m=B),
                            op=OP.is_equal)
                        for q in range(B):
                            k = bb * B + q
                            nc.tensor.matmul(
                                pa[:], lhsT=G[:, q * f_out:(q + 1) * f_out],
                                rhs=S[:, q * P:(q + 1) * P],
                                start=(k == 0), stop=(k == kc - 1))
                    sl = slice(t * P, (t + 1) * P)
                    nc.vector.tensor_tensor(out=u_tile[:, sl], in0=pa[:],
                                            in1=dinv_s[:, sl], op=OP.mult)

            def bn_stats_cols(v_ap, scol, qcol, idx):
                """free-dim sum and sq-sum of one [P, W] slab into col idx."""
                nc.vector.tensor_reduce(out=scol[:, idx:idx + 1], in_=v_ap,
                                        axis=mybir.AxisListType.X, op=OP.add)
                sq = wk.tile([P, GW], F32, tag="sq", name="sq")
                w = v_ap.shape[-1]
                nc.vector.tensor_tensor(out=sq[:, :w], in0=v_ap, in1=v_ap,
                                        op=OP.mult)
                nc.vector.tensor_reduce(out=qcol[:, idx:idx + 1],
                                        in_=sq[:, :w],
                                        axis=mybir.AxisListType.X, op=OP.add)

            def bn_allreduce(st, tag):
                """st: [P, 2*nfh] sums|sqsums -> AllReduce -> st2 tile."""
                w = st.shape[-1]
                stats_in = dpool.tile([P, w], F32, name=f"stats_in_{tag}")
                stats_out = dpool.tile([P, w], F32, addr_space="Shared",
                                       name=f"stats_out_{tag}")
                nc.sync.dma_start(stats_in[:], st[:])
                cc2 = nc.gpsimd.collective_compute(
                    "AllReduce", OP.add, replica_groups=RG,
                    ins=[stats_in[:].opt()], outs=[stats_out[:].opt()])
                st2 = wk.tile([P, w], F32, tag=f"st2_{tag}",
                              name=f"st2_{tag}")
                ld2 = nc.sync.dma_start(st2[:], stats_out[:])
                add_dep_helper(ld2.ins, cc2.ins, True, "stats after ar")
                return st2

            def bn_ab(st2, nfh, g_cols, be_cols, tag):
                """per-feature-half affine coefs A, B from reduced stats."""
                AB = []
                for fh in range(nfh):
                    mu = wk.tile([P, 1], F32, tag=f"mu{tag}{fh}",
                                 name=f"mu{tag}{fh}")
                    nc.vector.tensor_scalar(out=mu[:],
                                            in0=st2[:, fh:fh + 1],
                                            scalar1=1.0 / N, scalar2=None,
                                            op0=OP.mult)
                    var = wk.tile([P, 1], F32, tag=f"var{tag}{fh}",
                                  name=f"var{tag}{fh}")
                    nc.vector.tensor_tensor(out=var[:], in0=mu[:], in1=mu[:],
                                            op=OP.mult)
                    ms = wk.tile([P, 1], F32, tag=f"ms{tag}{fh}",
                                 name=f"ms{tag}{fh}")
                    nc.vector.tensor_scalar(
                        out=ms[:], in0=st2[:, nfh + fh:nfh + fh + 1],
                        scalar1=1.0 / N, scalar2=None, op0=OP.mult)
                    nc.vector.tensor_tensor(out=var[:], in0=ms[:], in1=var[:],
                                            op=OP.subtract)
                    nc.vector.tensor_scalar(out=var[:], in0=var[:],
                                            scalar1=EPS, scalar2=None,
                                            op0=OP.add)
                    rs = wk.tile([P, 1], F32, tag=f"rs{tag}{fh}",
                                 name=f"rs{tag}{fh}")
                    nc.scalar.activation(rs[:], var[:],
                                         mybir.ActivationFunctionType.Sqrt)
                    nc.vector.reciprocal(rs[:], rs[:])
                    A = wk.tile([P, 1], F32, tag=f"A{tag}{fh}",
                                name=f"A{tag}{fh}")
                    nc.vector.tensor_tensor(out=A[:], in0=g_cols[fh],
                                            in1=rs[:], op=OP.mult)
                    Bv = wk.tile([P, 1], F32, tag=f"B{tag}{fh}",
                                 name=f"B{tag}{fh}")
                    nc.vector.tensor_tensor(out=Bv[:], in0=mu[:], in1=A[:],
                                            op=OP.mult)
                    nc.vector.tensor_tensor(out=Bv[:], in0=be_cols[fh],
                                            in1=Bv[:], op=OP.subtract)
                    AB.append((A, Bv))
                return AB

            def affine_prelu(src_ap, dst_ap, A, Bv, a_col, w):
                t1 = wk.tile([P, GW], F32, tag="t1", name="t1")
                nc.vector.tensor_scalar(out=t1[:, :w], in0=src_ap,
                                        scalar1=A[:, :1], scalar2=Bv[:, :1],
                                        op0=OP.mult, op1=OP.add)
                t2 = wk.tile([P, GW], F32, tag="t2", name="t2")
                nc.vector.tensor_scalar(out=t2[:, :w], in0=t1[:, :w],
                                        scalar1=a_col, scalar2=None,
                                        op0=OP.mult)
                nc.vector.tensor_tensor(out=dst_ap, in0=t1[:, :w],
                                        in1=t2[:, :w], op=OP.max)

            # =================== layer 1 ==================================
            allgather(agin1, table1)
            u1 = hpool.tile([P, SHARD], BF16, tag="u1", name="u1")
            aggregate(table1, IN, u1)

            # v1 = W1^T u1 in 512-wide node groups; BN stats on the fly
            h1 = [hpool.tile([P, SHARD], BF16, tag=f"h1_{fh}",
                             name=f"h1_{fh}") for fh in range(2)]
            v1 = [hpool.tile([P, SHARD], BF16, tag=f"v1_{fh}",
                             name=f"v1_{fh}") for fh in range(2)]
            scol = [wk.tile([P, NG], F32, tag=f"sc{fh}", name=f"sc{fh}")
                    for fh in range(2)]
            qcol = [wk.tile([P, NG], F32, tag=f"qc{fh}", name=f"qc{fh}")
                    for fh in range(2)]
            for g in range(NG):
                gw = min(GW, SHARD - g * GW)
                sl = slice(g * GW, g * GW + gw)
                for fh in range(2):
                    pv = psv.tile([P, GW], F32, tag="pv", name="pv")
                    nc.tensor.matmul(pv[:, :gw],
                                     lhsT=w1_s[:, fh * P:(fh + 1) * P],
                                     rhs=u1[:, sl], start=True, stop=True)
                    nc.vector.tensor_copy(out=v1[fh][:, sl], in_=pv[:, :gw])
                    bn_stats_cols(v1[fh][:, sl], scol[fh], qcol[fh], g)
            st = wk.tile([P, 4], F32, tag="st1", name="st1")
            for fh in range(2):
                nc.vector.tensor_reduce(out=st[:, fh:fh + 1], in_=scol[fh][:],
                                        axis=mybir.AxisListType.X, op=OP.add)
                nc.vector.tensor_reduce(out=st[:, 2 + fh:3 + fh],
                                        in_=qcol[fh][:],
                                        axis=mybir.AxisListType.X, op=OP.add)
            st2 = bn_allreduce(st, "l1")
            AB1 = bn_ab(st2, 2, [gb_s[:, 0:1], gb_s[:, 1:2]],
                        [gb_s[:, 4:5], gb_s[:, 5:6]], "l1")
            for g in range(NG):
                gw = min(GW, SHARD - g * GW)
                sl = slice(g * GW, g * GW + gw)
                for fh in range(2):
                    affine_prelu(v1[fh][:, sl], h1[fh][:, sl],
                                 AB1[fh][0], AB1[fh][1], a_s[:, 0:1], gw)

            # =================== layer 2 ==================================
            for b in range(T):
                pm = psmm.tile([P, P], F32, tag="pm", name="pm")
                sl = slice(b * P, (b + 1) * P)
                nc.tensor.matmul(pm[:], lhsT=h1[0][:, sl], rhs=w2a_s[:],
                                 start=True, stop=False)
                nc.tensor.matmul(pm[:], lhsT=h1[1][:, sl], rhs=w2b_s[:],
                                 start=False, stop=True)
                tb = wk.tile([P, OUT], BF16, tag="tb2", name="tb2")
                nc.vector.tensor_scalar(out=tb[:], in0=pm[:],
                                        scalar1=ccol(2, b), scalar2=None,
                                        op0=OP.mult)
                nc.sync.dma_start(agin2[b * P:(b + 1) * P, :], tb[:])
            allgather(agin2, table2)
            v2 = hpool.tile([P, SHARD], BF16, tag="u1", name="v2")
            aggregate(table2, OUT, v2)
            scol2 = wk.tile([P, T], F32, tag="sc2", name="sc2")
            qcol2 = wk.tile([P, T], F32, tag="qc2", name="qc2")
            for t in range(T):
                bn_stats_cols(v2[:, t * P:(t + 1) * P], scol2, qcol2, t)
            st_2 = wk.tile([P, 2], F32, tag="st_2", name="st_2")
            nc.vector.tensor_reduce(out=st_2[:, 0:1], in_=scol2[:],
                                    axis=mybir.AxisListType.X, op=OP.add)
            nc.vector.tensor_reduce(out=st_2[:, 1:2], in_=qcol2[:],
                                    axis=mybir.AxisListType.X, op=OP.add)
            st2_2 = bn_allreduce(st_2, "l2")
            AB2 = bn_ab(st2_2, 1, [gb_s[:, 2:3]], [gb_s[:, 6:7]], "l2")
            h2 = hpool.tile([P, SHARD], BF16, tag="h2", name="h2")
            for t in range(T):
                sl = slice(t * P, (t + 1) * P)
                affine_prelu(v2[:, sl], h2[:, sl], AB2[0][0], AB2[0][1],
                             a_s[:, 1:2], P)

            # =================== layer 3 (decoder) ========================
            for b in range(T):
                pm = psmm.tile([P, P], F32, tag="pm", name="pm")
                sl = slice(b * P, (b + 1) * P)
                nc.tensor.matmul(pm[:], lhsT=h2[:, sl], rhs=wd_s[:],
                                 start=True, stop=True)
                tb = wk.tile([P, IN], BF16, tag="tb2", name="tb3")
                nc.vector.tensor_scalar(out=tb[:], in0=pm[:],
                                        scalar1=ccol(3, b), scalar2=None,
                                        op0=OP.mult)
                nc.sync.dma_start(agin3[b * P:(b + 1) * P, :], tb[:])
            allgather(agin3, table3)
            v3 = hpool.tile([P, SHARD], BF16, tag="h2", name="v3")
            aggregate(table3, IN, v3)
            scol3 = wk.tile([P, T], F32, tag="sc2", name="sc3")
            qcol3 = wk.tile([P, T], F32, tag="qc2", name="qc3")
            for t in range(T):
                bn_stats_cols(v3[:, t * P:(t + 1) * P], scol3, qcol3, t)
            st_3 = wk.tile([P, 2], F32, tag="st_2", name="st_3")
            nc.vector.tensor_reduce(out=st_3[:, 0:1], in_=scol3[:],
                                    axis=mybir.AxisListType.X, op=OP.add)
            nc.vector.tensor_reduce(out=st_3[:, 1:2], in_=qcol3[:],
                                    axis=mybir.AxisListType.X, op=OP.add)
            st2_3 = bn_allreduce(st_3, "l3")
            AB3 = bn_ab(st2_3, 1, [gb_s[:, 3:4]], [gb_s[:, 7:8]], "l3")
            rex = hpool.tile([P, SHARD], BF16, tag="u1", name="rex")
            for t in range(T):
                sl = slice(t * P, (t + 1) * P)
                affine_prelu(v3[:, sl], rex[:, sl], AB3[0][0], AB3[0][1],
                             a_s[:, 2:3], P)

            # =================== loss =====================================
            for b in range(T):
                ptb = pstr.tile([P, P], BF16, tag="ptrbf", name="ptrbf")
                nc.tensor.transpose(ptb[:], rex[:, b * P:(b + 1) * P],
                                    idbf_s[:])
                rn = wk.tile([P, P], BF16, tag="rn", name="rn")
                nc.vector.tensor_copy(out=rn[:], in_=ptb[:])
                nc.sync.dma_start(rex_dram[b * P:(b + 1) * P, :], rn[:])
            acc = cpool.tile([P, mc], F32, tag="acc", name="acc")
            for m in range(mc):
                pg = gp.tile([P, IN], BF16, tag="pg", name="pg")
                nc.gpsimd.indirect_dma_start(
                    out=pg[:], out_offset=None, in_=rex_dram[:],
                    in_offset=IndirectOffsetOnAxis(ap=mrl_s[:, m:m + 1],
                                                   axis=0))
                tg = gp.tile([P, IN], F32, tag="tg", name="tg")
                nc.gpsimd.indirect_dma_start(
                    out=tg[:], out_offset=None, in_=x_shard[:],
                    in_offset=IndirectOffsetOnAxis(ap=mrl_s[:, m:m + 1],
                                                   axis=0))
                pp = wk.tile([P, 1], F32, tag="pp", name="pp")
                tt = wk.tile([P, 1], F32, tag="tt", name="tt")
                ptv = wk.tile([P, 1], F32, tag="ptv", name="ptv")
                tmp = wk.tile([P, IN], F32, tag="tmp", name="tmp")
                nc.vector.tensor_tensor(out=tmp[:], in0=pg[:], in1=pg[:],
                                        op=OP.mult)
                nc.vector.tensor_reduce(out=pp[:], in_=tmp[:],
                                        axis=mybir.AxisListType.X, op=OP.add)
                nc.vector.tensor_tensor(out=tmp[:], in0=tg[:], in1=tg[:],
                                        op=OP.mult)
                nc.vector.tensor_reduce(out=tt[:], in_=tmp[:],
                                        axis=mybir.AxisListType.X, op=OP.add)
                nc.vector.tensor_tensor(out=tmp[:], in0=pg[:], in1=tg[:],
                                        op=OP.mult)
                nc.vector.tensor_reduce(out=ptv[:], in_=tmp[:],
                                        axis=mybir.AxisListType.X, op=OP.add)
                q = wk.tile([P, 1], F32, tag="q", name="q")
                nc.vector.tensor_tensor(out=q[:], in0=pp[:], in1=tt[:],
                                        op=OP.mult)
                nc.vector.tensor_scalar(out=q[:], in0=q[:], scalar1=1e-30,
                                        scalar2=None, op0=OP.add)
                rq = wk.tile([P, 1], F32, tag="rq", name="rq")
                nc.scalar.activation(rq[:], q[:],
                                     mybir.ActivationFunctionType.Sqrt)
                nc.vector.reciprocal(rq[:], rq[:])
                nc.vector.tensor_tensor(out=rq[:], in0=ptv[:], in1=rq[:],
                                        op=OP.mult)
                nc.vector.tensor_tensor(out=acc[:, m:m + 1], in0=rq[:],
                                        in1=mv_s[:, m:m + 1], op=OP.mult)
            accr = wk.tile([P, 1], F32, tag="accr", name="accr")
            nc.vector.tensor_reduce(out=accr[:], in_=acc[:],
                                    axis=mybir.AxisListType.X, op=OP.add)
            pl = pstr.tile([1, 1], F32, tag="ptr32", name="ptr32")
            nc.tensor.matmul(pl[:], lhsT=accr[:], rhs=ones_s[:], start=True,
                             stop=True)
            lsb = wk.tile([1, 16], F32, tag="lsb", name="lsb")
            nc.gpsimd.memset(lsb[:], 0.0)
            nc.vector.tensor_copy(out=lsb[:, 0:1], in_=pl[:])
            nc.sync.dma_start(loss_in[:], lsb[:])
            cc3 = nc.gpsimd.collective_compute(
                "AllReduce", OP.add, replica_groups=RG,
                ins=[loss_in[:].opt()], outs=[loss_out[:].opt()])
            lsum = wk.tile([1, 16], F32, tag="lsum", name="lsum")
            ld3 = nc.sync.dma_start(lsum[:], loss_out[:])
            add_dep_helper(ld3.ins, cc3.ins, True, "loss after ar")
            nc.vector.tensor_scalar(out=lsb[:, 0:1], in0=lsum[:, 0:1],
                                    scalar1=-1.0 / NMASK, scalar2=1.0,
                                    op0=OP.mult, op1=OP.add)
            nc.sync.dma_start(out_t[:], lsb[:, 0:1])
    return nc


def prepare(inputs):
    inputs = {k: np.asarray(v) for k, v in inputs.items()}
    edge_index = inputs["edge_index"].astype(np.int64)
    mask_nodes = inputs["mask_nodes"].astype(np.int64)
    x = inputs["x"].astype(np.float32)
    (dinv_pad, srcs, dstl, kc, mlocal, mvalid, mkeep, mc) = _prep(
        edge_index, mask_nodes)

    nc = build_nc(kc, mc)

    iota_b = np.broadcast_to(
        np.broadcast_to(np.arange(P, dtype=np.float32), (B, P)).reshape(
            1, B * P), (P, B * P)).astype(BF)
    ident_bf = np.eye(P, dtype=np.float32).astype(BF)
    gb = np.zeros((P, 8), np.float32)
    gb[:, 0] = inputs["g1"][:P]
    gb[:, 1] = inputs["g1"][P:]
    gb[:, 2] = inputs["g2"]
    gb[:, 3] = inputs["gd"]
    gb[:, 4] = inputs["be1"][:P]
    gb[:, 5] = inputs["be1"][P:]
    gb[:, 6] = inputs["be2"]
    gb[:, 7] = inputs["bed"]
    a_rep = np.zeros((P, 3), np.float32)
    a_rep[:, 0] = inputs["a1"][0]
    a_rep[:, 1] = inputs["a2"][0]
    a_rep[:, 2] = inputs["ad"][0]
    w1 = inputs["W1"].astype(BF)
    w2 = inputs["W2"].astype(BF)
    wd = inputs["Wd"].astype(BF)
    token = inputs["mask_token"].astype(np.float32)

    in_maps = []
    for c in range(NC):
        rows = np.arange(c * SHARD, (c + 1) * SHARD)
        xs = np.zeros((SHARD, IN), np.float32)
        v = rows < N
        xs[v] = x[rows[v]]
        dloc = dinv_pad[c * SHARD:(c + 1) * SHARD]
        keep = mkeep[c]
        cols = np.zeros((P, 4 * T), np.float32)
        cols[:, 0 * T:1 * T] = (keep * dloc).reshape(T, P).T
        cols[:, 1 * T:2 * T] = ((1.0 - keep) * dloc).reshape(T, P).T
        cols[:, 2 * T:3 * T] = dloc.reshape(T, P).T
        cols[:, 3 * T:4 * T] = (keep * dloc).reshape(T, P).T
        in_maps.append({
            "x_shard": xs,
            "src_rows": srcs[c],
            "dst_local": dstl[c],
            "dinv_row": np.ascontiguousarray(dloc[None, :]),
            "cols": cols,
            "token_rep": np.ascontiguousarray(
                np.broadcast_to(token[None, :], (P, IN))),
            "iota_b": np.ascontiguousarray(iota_b),
            "ident_bf": ident_bf,
            "ones_col": np.ones((P, 1), np.float32),
            "w1": w1,
            "w2a": np.ascontiguousarray(w2[:P]),
            "w2b": np.ascontiguousarray(w2[P:]),
            "wd": wd,
            "gb": gb,
            "a_rep": a_rep,
            "mrow_l": mlocal[c],
            "mval": mvalid[c],
        })
    return nc, in_maps


def kernel(**inputs):
    import os
    nc, in_maps = prepare(inputs)
    res = run_bass_kernel_spmd(nc, in_maps, core_ids=list(range(NC)),
                               trace=bool(os.environ.get("KTRACE")))
    kernel._last_results = res
    loss = res.results[0]["loss"][0, 0]
    return np.float32(loss).reshape(())


# revision 27
# speedup vs baseline: 25.1921x; 25.1921x over previous
"""GraphMAE-style GNN forward (3x GCNConv + BN + PReLU + SCE loss) on 8 TRN2
NeuronCores via Bass/Tile — v3, optimized for wall-clock (host+compile+exec).

Dataflow (per core, nodes sharded 6272/core, feature-major activations):
  table0 = dinv * x-masked (node-major bf16, built straight from x_shard)
  AllGather -> agg (indirect row gathers + one-hot segment-sum matmuls with
  self-loops folded into the edge list as ordinary edges) -> u1 [128,SHARD]
  -> v1 = W1^T u1 (feature-major, 512-wide rhs groups) -> BN(AllReduce)+PReLU
  -> h1.  L2/L3: table = dinv*(h W) node-major via lhsT=h matmuls (no
  transposes), AllGather, aggregate (128 feats), BN+PReLU.  Loss: per-core
  masked-row cosine partial sums + AllReduce.

Conv biases are skipped: training-mode BatchNorm cancels them exactly.
"""
import sys

sys.path.insert(0, "/opt/trn_rl_repo")
import numpy as np
import ml_dtypes

import concourse.bass as bass
import concourse.mybir as mybir
import concourse.tile as tile
from bass_rust import SyncInfo
from concourse.bass import IndirectOffsetOnAxis
from concourse.bass_utils import run_bass_kernel_spmd
from concourse.tile_rust import add_dep_helper
from concourse.vector_clock import ScopedClock

BF = ml_dtypes.bfloat16
F32 = mybir.dt.float32
BF16 = mybir.dt.bfloat16
I32 = mybir.dt.int32
OP = mybir.AluOpType

N, E, IN, HID, OUT = 50000, 800000, 128, 256, 128
NC, P = 8, 128
SHARD, T = 6272, 49
NPAD = NC * SHARD
NMASK = 25000
EPS = 1e-5
RG = [list(range(NC))]
B = 18         # chunks per batched one-hot build (= kc: one per dst tile)
GW = 512       # node-group width for the feature-major W matmuls
SW = 896       # slab width for BN stats / affine passes (7 tiles)

# ---------------------------------------------------------------------------
# Walrus on this toolchain rejects >1 semaphore wait per instruction
# ("Too many sync wait commands"). Post-process lowered blocks: hoist excess
# waits onto injected same-engine NoOps (program order => equivalent).
_uid = [0]


def _split_bb_waits(nc):
    for f in nc.m.functions:
        for bb in f.blocks:
            insts = list(bb.instructions)
            out = []
            changed = False
            for inst in insts:
                si = inst.sync_info
                waits = list(si.on_wait) if si is not None and si.on_wait else []
                if len(waits) > 1:
                    changed = True
                    rest = waits[:-1]
                    inst.sync_info.on_wait = waits[-1:]
                    while rest:
                        _uid[0] += 1
                        nop = mybir.InstNoOp(
                            name=f"I-waitsplit-{_uid[0]}", ins=[], outs=[])
                        nop.engine = inst.engine
                        nop.sync_info = SyncInfo(
                            on_wait=rest[:1], on_update=[])
                        rest = rest[1:]
                        out.append(nop)
                out.append(inst)
            if changed:
                bb.instructions = out


class TileContextSplitDrain(tile.TileContext):
    def _drain_and_barrier(self, tick_clock, wait_clock):
        nc = self.nc
        probe = nc.sync.nop(nofuse=True)
        wait_clock.add_sem_waits(
            probe.ins, ScopedClock({None: tick_clock.global_clock}))
        nc.sync.drain()
        nc.all_engine_barrier()
        assert self.sems is not None
        popped = nc._tile_sem_poison_stack.pop()
        assert popped is self._sem_poison
        nc.clear_and_free_semaphores(list(self.sems.allocated().values()))
        nc.all_engine_barrier()
        _split_bb_waits(nc)


# ---------------------------------------------------------------------------
def _prep(edge_index, mask_nodes):
    src = edge_index[0].astype(np.int64)
    dst = edge_index[1].astype(np.int64)
    deg = np.bincount(dst, minlength=N).astype(np.float64) + 1.0
    dinv = (1.0 / np.sqrt(deg)).astype(np.float32)
    dinv_pad = np.zeros(NPAD, np.float32)
    dinv_pad[:N] = dinv

    # fold self-loops in as ordinary edges (table rows carry dinv_src, the
    # post-aggregation dinv_dst factor supplies the other dinv)
    loops = np.arange(N, dtype=np.int64)
    src = np.concatenate([src, loops])
    dst = np.concatenate([dst, loops])

    mask_set = np.zeros(N, bool)
    mask_set[mask_nodes] = True

    pertile = []
    kc = 0
    core_of = dst // SHARD
    for c in range(NC):
        sel = core_of == c
        es, ed = src[sel], dst[sel] - c * SHARD
        tl = ed // P
        order = np.argsort(tl, kind="stable")
        es, ed, tl = es[order], ed[order], tl[order]
        tiles = []
        for t in range(T):
            m = tl == t
            tiles.append((es[m], (ed[m] % P).astype(np.float32)))
            kc = max(kc, (int(m.sum()) + P - 1) // P)
        pertile.append(tiles)

    srcs, dstl = [], []
    for c in range(NC):
        sc = np.zeros((T * kc, P), np.int32)
        dc = np.full((T * kc, P), 255.0, np.float32)
        for t in range(T):
            es, dl = pertile[c][t]
            n = len(es)
            sc[t * kc:(t + 1) * kc].reshape(-1)[:n] = es
            dc[t * kc:(t + 1) * kc].reshape(-1)[:n] = dl
        srcs.append(np.ascontiguousarray(sc.T))
        dstl.append(np.ascontiguousarray(dc.T.astype(BF)))

    mlocal, mvalid, mkeep = [], [], []
    locs = []
    mc = 0
    for c in range(NC):
        rows = np.arange(c * SHARD, (c + 1) * SHARD)
        valid = rows < N
        mm = np.zeros(SHARD, bool)
        mm[:valid.sum()] = mask_set[rows[valid]]
        loc = np.where(mm)[0]
        mc = max(mc, (len(loc) + P - 1) // P)
        locs.append(loc)
        keep = np.ones(SHARD, np.float32)
        keep[mm] = 0.0
        mkeep.append(keep)
    for c in range(NC):
        loc = locs[c]
        n = len(loc)
        lo = np.zeros((mc, P), np.int32)
        va = np.zeros((mc, P), np.float32)
        lo.reshape(-1)[:n] = loc
        va.reshape(-1)[:n] = 1.0
        mlocal.append(np.ascontiguousarray(lo.T))
        mvalid.append(np.ascontiguousarray(va.T))
    return dinv_pad, srcs, dstl, kc, mlocal, mvalid, mkeep, mc


def build_nc(kc, mc):
    nc = bass.Bass(num_devices=NC, target_bir_lowering=False)
    D = {}

    def inp(name, shape, dt):
        D[name] = nc.dram_tensor(name, shape, dt, kind="ExternalInput")
        return D[name]

    x_shard = inp("x_shard", [SHARD, IN], BF16)
    inp("src_rows", [P, T * kc], I32)
    inp("dst_local", [P, T * kc], BF16)
    inp("dinv_row", [1, SHARD], F32)
    inp("cols", [P, 4 * T], F32)   # l1x | l1tok | d | l3 blocks of T
    inp("token_rep", [P, IN], F32)
    inp("iota_row", [1, B * P], BF16)
    inp("ident_bf", [P, P], BF16)
    inp("ones_col", [P, 1], F32)
    inp("w1", [IN, HID], BF16)
    inp("w2a", [P, OUT], BF16)
    inp("w2b", [P, OUT], BF16)
    inp("wd", [OUT, IN], BF16)
    inp("gb", [P, 8], F32)
    inp("a_rep", [P, 3], F32)
    inp("mrow_l", [P, mc], I32)
    inp("mval", [P, mc], F32)
    out_t = nc.dram_tensor("loss", [1, 1], F32, kind="ExternalOutput")

    NB = kc // B           # one-hot batches per dst tile
    NG = (SHARD + GW - 1) // GW   # node groups for L1 W matmul

    with TileContextSplitDrain(nc) as tc:
        with (
            tc.tile_pool(name="const", bufs=1) as cpool,
            tc.tile_pool(name="hbuf", bufs=1) as hpool,
            tc.tile_pool(name="work", bufs=2) as wk,
            tc.tile_pool(name="gpool", bufs=4) as gp,
            tc.tile_pool(name="psmm", bufs=2, space="PSUM") as psmm,
            tc.tile_pool(name="psv", bufs=2, space="PSUM") as psv,
            tc.tile_pool(name="pstr", bufs=1, space="PSUM") as pstr,
            tc.tile_pool(name="psagg", bufs=2, space="PSUM") as psagg,
            tc.tile_pool(name="dram", bufs=1, space="DRAM") as dpool,
        ):
            def load(tname):
                h = D[tname]
                t_ = cpool.tile(list(h.shape), h.dtype, tag=tname, name=tname)
                nc.sync.dma_start(t_[:], h[:])
                return t_

            src_s = load("src_rows")
            dstl_s = load("dst_local")
            cols_s = load("cols")
            tokr_s = load("token_rep")
            iota_s = cpool.tile([P, B * P], BF16, tag="iota_b", name="iota_b")
            nc.sync.dma_start(iota_s[:],
                              D["iota_row"][0:1, :].to_broadcast([P, B * P]))
            idbf_s = load("ident_bf")
            ones_s = load("ones_col")
            w1_s = load("w1")
            w2a_s = load("w2a")
            w2b_s = load("w2b")
            wd_s = load("wd")
            gb_s = load("gb")
            a_s = load("a_rep")
            mrl_s = load("mrow_l")
            mv_s = load("mval")
            dinv_s = cpool.tile([P, SHARD], F32, tag="dinv_rep",
                                name="dinv_rep")
            nc.sync.dma_start(dinv_s[:],
                              D["dinv_row"][0:1, :].to_broadcast([P, SHARD]))

            def ccol(blk, b):
                return cols_s[:, blk * T + b:blk * T + b + 1]

            table1 = dpool.tile([NPAD, IN], BF16, addr_space="Shared",
                                name="table1")
            table2 = dpool.tile([NPAD, OUT], BF16, addr_space="Shared",
                                name="table2")
            table3 = dpool.tile([NPAD, IN], BF16, addr_space="Shared",
                                name="table3")
            agin1 = dpool.tile([SHARD, IN], BF16, name="agin1")
            agin2 = dpool.tile([SHARD, OUT], BF16, name="agin2")
            agin3 = dpool.tile([SHARD, IN], BF16, name="agin3")
            loss_in = dpool.tile([1, 16], F32, name="loss_in")
            loss_out = dpool.tile([1, 16], F32, addr_space="Shared",
                                  name="loss_out")
            rex_dram = dpool.tile([SHARD, IN], BF16, name="rex_dram")

            # ---- table0: dinv * masked-x, node-major bf16 ----------------
            for b in range(T):
                xt = wk.tile([P, IN], BF16, tag="xt", name="xt")
                nc.sync.dma_start(xt[:], x_shard[b * P:(b + 1) * P, :])
                tb = wk.tile([P, IN], BF16, tag="tb0", name="tb0")
                tk = wk.tile([P, IN], F32, tag="tk0", name="tk0")
                nc.vector.tensor_scalar(out=tk[:], in0=tokr_s[:],
                                        scalar1=ccol(1, b), scalar2=None,
                                        op0=OP.mult)
                xs2 = wk.tile([P, IN], F32, tag="xs2", name="xs2")
                nc.vector.tensor_scalar(out=xs2[:], in0=xt[:],
                                        scalar1=ccol(0, b), scalar2=None,
                                        op0=OP.mult)
                nc.vector.tensor_tensor(out=tb[:], in0=xs2[:], in1=tk[:],
                                        op=OP.add)
                nc.sync.dma_start(agin1[b * P:(b + 1) * P, :], tb[:])

            def allgather(agin, table):
                cc = nc.gpsimd.collective_compute(
                    "AllGather", OP.bypass, replica_groups=RG,
                    ins=[agin[:].opt()], outs=[table[:].opt()])
                fence = nc.gpsimd.memset(
                    wk.tile([1, 1], F32, tag="fence", name="fence")[:], 0.0)
                add_dep_helper(fence.ins, cc.ins, True, "fence cc")

            def aggregate(table, f_out, u_tile):
                """u = dinv_dst * sum_edges table[src], feature-major bf16."""
                for t in range(T):
                    pa = psagg.tile([P, P], F32, tag="pa", name="pa")
                    for b0 in range(0, kc, B):
                        bw = min(B, kc - b0)
                        j0 = t * kc + b0
                        G = gp.tile([P, B * f_out], BF16, tag="G", name="G")
                        for q in range(bw):
                            nc.gpsimd.indirect_dma_start(
                                out=G[:, q * f_out:(q + 1) * f_out],
                                out_offset=None, in_=table[:],
                                in_offset=IndirectOffsetOnAxis(
                                    ap=src_s[:, j0 + q:j0 + q + 1], axis=0))
                        S = gp.tile([P, B * P], BF16, tag="S", name="S")
                        nc.vector.tensor_tensor(
                            out=S[:, :bw * P].rearrange(
                                "p (m q) -> p m q", m=bw),
                            in0=dstl_s[:, j0:j0 + bw].unsqueeze(2)
                                .to_broadcast([P, bw, P]),
                            in1=iota_s[:, :bw * P].rearrange(
                                "p (m q) -> p m q", m=bw),
                            op=OP.is_equal)
                        for q in range(bw):
                            k = b0 + q
                            nc.tensor.matmul(
                                pa[:], lhsT=G[:, q * f_out:(q + 1) * f_out],
                                rhs=S[:, q * P:(q + 1) * P],
                                start=(k == 0), stop=(k == kc - 1))
                    sl = slice(t * P, (t + 1) * P)
                    nc.vector.tensor_tensor(out=u_tile[:, sl], in0=pa[:],
                                            in1=dinv_s[:, sl], op=OP.mult)

            def bn_stats(v_tile, st, scol_tag):
                """sum and sq-sum over a [P, SHARD] tile -> st cols 0/1 (or
                given slices): whole-tile 3D reduce for the sum, slab-wise
                squares for the sq-sum."""
                s_all, q_all = st
                nc.vector.tensor_reduce(
                    out=s_all, in_=v_tile[:].rearrange(
                        "p (a b) -> p a b", b=P),
                    axis=mybir.AxisListType.XY, op=OP.add)
                qcol = wk.tile([P, T], F32, tag=scol_tag, name=scol_tag)
                nsl = SHARD // SW
                for g in range(nsl):
                    sl = slice(g * SW, (g + 1) * SW)
                    sq = wk.tile([P, SW], F32, tag="sq", name="sq")
                    nc.vector.tensor_tensor(out=sq[:], in0=v_tile[:, sl],
                                            in1=v_tile[:, sl], op=OP.mult)
                    nc.vector.tensor_reduce(
                        out=qcol[:, g * (SW // P):(g + 1) * (SW // P)],
                        in_=sq[:].rearrange("p (a b) -> p a b", b=P),
                        axis=mybir.AxisListType.X, op=OP.add)
                nc.vector.tensor_reduce(out=q_all, in_=qcol[:],
                                        axis=mybir.AxisListType.X, op=OP.add)

            def bn_allreduce(st, tag):
                """st: [P, 2*nfh] sums|sqsums -> AllReduce -> st2 tile."""
                w = st.shape[-1]
                stats_in = dpool.tile([P, w], F32, name=f"stats_in_{tag}")
                stats_out = dpool.tile([P, w], F32, addr_space="Shared",
                                       name=f"stats_out_{tag}")
                nc.sync.dma_start(stats_in[:], st[:])
                cc2 = nc.gpsimd.collective_compute(
                    "AllReduce", OP.add, replica_groups=RG,
                    ins=[stats_in[:].opt()], outs=[stats_out[:].opt()])
                st2 = wk.tile([P, w], F32, tag=f"st2_{tag}",
                              name=f"st2_{tag}")
                ld2 = nc.sync.dma_start(st2[:], stats_out[:])
                add_dep_helper(ld2.ins, cc2.ins, True, "stats after ar")
                return st2

            def bn_ab(st2, nfh, g_cols, be_cols, tag):
                """per-feature-half affine coefs A, B from reduced stats."""
                AB = []
                for fh in range(nfh):
                    mu = wk.tile([P, 1], F32, tag=f"mu{tag}{fh}",
                                 name=f"mu{tag}{fh}")
                    nc.vector.tensor_scalar(out=mu[:],
                                            in0=st2[:, fh:fh + 1],
                                            scalar1=1.0 / N, scalar2=None,
                                            op0=OP.mult)
                    var = wk.tile([P, 1], F32, tag=f"var{tag}{fh}",
                                  name=f"var{tag}{fh}")
                    nc.vector.tensor_tensor(out=var[:], in0=mu[:], in1=mu[:],
                                            op=OP.mult)
                    ms = wk.tile([P, 1], F32, tag=f"ms{tag}{fh}",
                                 name=f"ms{tag}{fh}")
                    nc.vector.tensor_scalar(
                        out=ms[:], in0=st2[:, nfh + fh:nfh + fh + 1],
                        scalar1=1.0 / N, scalar2=None, op0=OP.mult)
                    nc.vector.tensor_tensor(out=var[:], in0=ms[:], in1=var[:],
                                            op=OP.subtract)
                    nc.vector.tensor_scalar(out=var[:], in0=var[:],
                                            scalar1=EPS, scalar2=None,
                                            op0=OP.add)
                    rs = wk.tile([P, 1], F32, tag=f"rs{tag}{fh}",
                                 name=f"rs{tag}{fh}")
                    nc.scalar.activation(rs[:], var[:],
                                         mybir.ActivationFunctionType.Sqrt)
                    nc.vector.reciprocal(rs[:], rs[:])
                    A = wk.tile([P, 1], F32, tag=f"A{tag}{fh}",
                                name=f"A{tag}{fh}")
                    nc.vector.tensor_tensor(out=A[:], in0=g_cols[fh],
                                            in1=rs[:], op=OP.mult)
                    Bv = wk.tile([P, 1], F32, tag=f"B{tag}{fh}",
                                 name=f"B{tag}{fh}")
                    nc.vector.tensor_tensor(out=Bv[:], in0=mu[:], in1=A[:],
                                            op=OP.mult)
                    nc.vector.tensor_tensor(out=Bv[:], in0=be_cols[fh],
                                            in1=Bv[:], op=OP.subtract)
                    AB.append((A, Bv))
                return AB

            def affine_prelu(src_ap, dst_ap, A, Bv, a_col, w):
                t1 = wk.tile([P, SW], F32, tag="t1", name="t1")
                nc.vector.tensor_scalar(out=t1[:, :w], in0=src_ap,
                                        scalar1=A[:, :1], scalar2=Bv[:, :1],
                                        op0=OP.mult, op1=OP.add)
                t2 = wk.tile([P, SW], F32, tag="t2", name="t2")
                nc.vector.tensor_scalar(out=t2[:, :w], in0=t1[:, :w],
                                        scalar1=a_col, scalar2=None,
                                        op0=OP.mult)
                nc.vector.tensor_tensor(out=dst_ap, in0=t1[:, :w],
                                        in1=t2[:, :w], op=OP.max)

            def affine_prelu_full(v_tile, h_tile, A, Bv, a_col):
                for g in range(SHARD // SW):
                    sl = slice(g * SW, (g + 1) * SW)
                    affine_prelu(v_tile[:, sl], h_tile[:, sl], A, Bv,
                                 a_col, SW)

            # =================== layer 1 ==================================
            allgather(agin1, table1)
            u1 = hpool.tile([P, SHARD], BF16, tag="u1", name="u1")
            aggregate(table1, IN, u1)

            # v1 = W1^T u1 in 512-wide node groups; BN stats on the fly
            h1 = [hpool.tile([P, SHARD], BF16, tag=f"h1_{fh}",
                             name=f"h1_{fh}") for fh in range(2)]
            v1 = [hpool.tile([P, SHARD], BF16, tag=f"v1_{fh}",
                             name=f"v1_{fh}") for fh in range(2)]
            for g in range(NG):
                gw = min(GW, SHARD - g * GW)
                sl = slice(g * GW, g * GW + gw)
                for fh in range(2):
                    pv = psv.tile([P, GW], F32, tag="pv", name="pv")
                    nc.tensor.matmul(pv[:, :gw],
                                     lhsT=w1_s[:, fh * P:(fh + 1) * P],
                                     rhs=u1[:, sl], start=True, stop=True)
                    nc.vector.tensor_copy(out=v1[fh][:, sl], in_=pv[:, :gw])
            st = wk.tile([P, 4], F32, tag="st1", name="st1")
            for fh in range(2):
                bn_stats(v1[fh], (st[:, fh:fh + 1], st[:, 2 + fh:3 + fh]),
                         f"qc1{fh}")
            st2 = bn_allreduce(st, "l1")
            AB1 = bn_ab(st2, 2, [gb_s[:, 0:1], gb_s[:, 1:2]],
                        [gb_s[:, 4:5], gb_s[:, 5:6]], "l1")
            for fh in range(2):
                affine_prelu_full(v1[fh], h1[fh], AB1[fh][0], AB1[fh][1],
                                  a_s[:, 0:1])

            # =================== layer 2 ==================================
            for b in range(T):
                pm = psmm.tile([P, P], F32, tag="pm", name="pm")
                sl = slice(b * P, (b + 1) * P)
                nc.tensor.matmul(pm[:], lhsT=h1[0][:, sl], rhs=w2a_s[:],
                                 start=True, stop=False)
                nc.tensor.matmul(pm[:], lhsT=h1[1][:, sl], rhs=w2b_s[:],
                                 start=False, stop=True)
                tb = wk.tile([P, OUT], BF16, tag="tb2", name="tb2")
                nc.vector.tensor_scalar(out=tb[:], in0=pm[:],
                                        scalar1=ccol(2, b), scalar2=None,
                                        op0=OP.mult)
                nc.sync.dma_start(agin2[b * P:(b + 1) * P, :], tb[:])
            allgather(agin2, table2)
            v2 = hpool.tile([P, SHARD], BF16, tag="u1", name="v2")
            aggregate(table2, OUT, v2)
            st_2 = wk.tile([P, 2], F32, tag="st_2", name="st_2")
            bn_stats(v2, (st_2[:, 0:1], st_2[:, 1:2]), "qc2")
            st2_2 = bn_allreduce(st_2, "l2")
            AB2 = bn_ab(st2_2, 1, [gb_s[:, 2:3]], [gb_s[:, 6:7]], "l2")
            h2 = hpool.tile([P, SHARD], BF16, tag="h2", name="h2")
            affine_prelu_full(v2, h2, AB2[0][0], AB2[0][1], a_s[:, 1:2])

            # =================== layer 3 (decoder) ========================
            for b in range(T):
                pm = psmm.tile([P, P], F32, tag="pm", name="pm")
                sl = slice(b * P, (b + 1) * P)
                nc.tensor.matmul(pm[:], lhsT=h2[:, sl], rhs=wd_s[:],
                                 start=True, stop=True)
                tb = wk.tile([P, IN], BF16, tag="tb2", name="tb3")
                nc.vector.tensor_scalar(out=tb[:], in0=pm[:],
                                        scalar1=ccol(3, b), scalar2=None,
                                        op0=OP.mult)
                nc.sync.dma_start(agin3[b * P:(b + 1) * P, :], tb[:])
            allgather(agin3, table3)
            v3 = hpool.tile([P, SHARD], BF16, tag="h2", name="v3")
            aggregate(table3, IN, v3)
            st_3 = wk.tile([P, 2], F32, tag="st_2", name="st_3")
            bn_stats(v3, (st_3[:, 0:1], st_3[:, 1:2]), "qc2")
            st2_3 = bn_allreduce(st_3, "l3")
            AB3 = bn_ab(st2_3, 1, [gb_s[:, 3:4]], [gb_s[:, 7:8]], "l3")
            rex = hpool.tile([P, SHARD], BF16, tag="u1", name="rex")
            affine_prelu_full(v3, rex, AB3[0][0], AB3[0][1], a_s[:, 2:3])

            # =================== loss =====================================
            for b in range(T):
                ptb = pstr.tile([P, P], BF16, tag="ptrbf", name="ptrbf")
                nc.tensor.transpose(ptb[:], rex[:, b * P:(b + 1) * P],
                                    idbf_s[:])
                rn = wk.tile([P, P], BF16, tag="rn", name="rn")
                nc.vector.tensor_copy(out=rn[:], in_=ptb[:])
                nc.sync.dma_start(rex_dram[b * P:(b + 1) * P, :], rn[:])
            pg_all = cpool.tile([P, mc * IN], BF16, tag="pg_all",
                                name="pg_all")
            tg_all = cpool.tile([P, mc * IN], BF16, tag="tg_all",
                                name="tg_all")
            for m in range(mc):
                nc.gpsimd.indirect_dma_start(
                    out=pg_all[:, m * IN:(m + 1) * IN], out_offset=None,
                    in_=rex_dram[:],
                    in_offset=IndirectOffsetOnAxis(ap=mrl_s[:, m:m + 1],
                                                   axis=0))
                nc.gpsimd.indirect_dma_start(
                    out=tg_all[:, m * IN:(m + 1) * IN], out_offset=None,
                    in_=x_shard[:],
                    in_offset=IndirectOffsetOnAxis(ap=mrl_s[:, m:m + 1],
                                                   axis=0))
            ltmp = cpool.tile([P, mc * IN], F32, tag="ltmp", name="ltmp")
            pp = wk.tile([P, mc], F32, tag="pp", name="pp")
            tt = wk.tile([P, mc], F32, tag="tt", name="tt")
            ptv = wk.tile([P, mc], F32, tag="ptv", name="ptv")
            for dst, in0, in1 in ((pp, pg_all, pg_all),
                                  (tt, tg_all, tg_all),
                                  (ptv, pg_all, tg_all)):
                nc.vector.tensor_tensor(out=ltmp[:], in0=in0[:], in1=in1[:],
                                        op=OP.mult)
                nc.vector.tensor_reduce(
                    out=dst[:], in_=ltmp[:].rearrange(
                        "p (a b) -> p a b", b=IN),
                    axis=mybir.AxisListType.X, op=OP.add)
            q = wk.tile([P, mc], F32, tag="q", name="q")
            nc.vector.tensor_tensor(out=q[:], in0=pp[:], in1=tt[:],
                                    op=OP.mult)
            nc.vector.tensor_scalar(out=q[:], in0=q[:], scalar1=1e-30,
                                    scalar2=None, op0=OP.add)
            nc.scalar.activation(q[:], q[:],
                                 mybir.ActivationFunctionType.Sqrt)
            nc.vector.reciprocal(q[:], q[:])
            nc.vector.tensor_tensor(out=q[:], in0=ptv[:], in1=q[:],
                                    op=OP.mult)
            nc.vector.tensor_tensor(out=q[:], in0=q[:], in1=mv_s[:],
                                    op=OP.mult)
            accr = wk.tile([P, 1], F32, tag="accr", name="accr")
            nc.vector.tensor_reduce(out=accr[:], in_=q[:],
                                    axis=mybir.AxisListType.X, op=OP.add)
            pl = pstr.tile([1, 1], F32, tag="ptr32", name="ptr32")
            nc.tensor.matmul(pl[:], lhsT=accr[:], rhs=ones_s[:], start=True,
                             stop=True)
            lsb = wk.tile([1, 16], F32, tag="lsb", name="lsb")
            nc.gpsimd.memset(lsb[:], 0.0)
            nc.vector.tensor_copy(out=lsb[:, 0:1], in_=pl[:])
            nc.sync.dma_start(loss_in[:], lsb[:])
            cc3 = nc.gpsimd.collective_compute(
                "AllReduce", OP.add, replica_groups=RG,
                ins=[loss_in[:].opt()], outs=[loss_out[:].opt()])
            lsum = wk.tile([1, 16], F32, tag="lsum", name="lsum")
            ld3 = nc.sync.dma_start(lsum[:], loss_out[:])
            add_dep_helper(ld3.ins, cc3.ins, True, "loss after ar")
            nc.vector.tensor_scalar(out=lsb[:, 0:1], in0=lsum[:, 0:1],
                                    scalar1=-1.0 / NMASK, scalar2=1.0,
                                    op0=OP.mult, op1=OP.add)
            nc.sync.dma_start(out_t[:], lsb[:, 0:1])
    return nc


def prepare(inputs):
    inputs = {k: np.asarray(v) for k, v in inputs.items()}
    edge_index = inputs["edge_index"].astype(np.int64)
    mask_nodes = inputs["mask_nodes"].astype(np.int64)
    x = inputs["x"].astype(np.float32)
    (dinv_pad, srcs, dstl, kc, mlocal, mvalid, mkeep, mc) = _prep(
        edge_index, mask_nodes)

    nc = build_nc(kc, mc)

    ident_bf = np.eye(P, dtype=np.float32).astype(BF)
    gb = np.zeros((P, 8), np.float32)
    gb[:, 0] = inputs["g1"][:P]
    gb[:, 1] = inputs["g1"][P:]
    gb[:, 2] = inputs["g2"]
    gb[:, 3] = inputs["gd"]
    gb[:, 4] = inputs["be1"][:P]
    gb[:, 5] = inputs["be1"][P:]
    gb[:, 6] = inputs["be2"]
    gb[:, 7] = inputs["bed"]
    a_rep = np.zeros((P, 3), np.float32)
    a_rep[:, 0] = inputs["a1"][0]
    a_rep[:, 1] = inputs["a2"][0]
    a_rep[:, 2] = inputs["ad"][0]
    w1 = inputs["W1"].astype(BF)
    w2 = inputs["W2"].astype(BF)
    wd = inputs["Wd"].astype(BF)
    token = inputs["mask_token"].astype(np.float32)

    in_maps = []
    for c in range(NC):
        rows = np.arange(c * SHARD, (c + 1) * SHARD)
        xs = np.zeros((SHARD, IN), BF)
        v = rows < N
        xs[v] = x[rows[v]].astype(BF)
        dloc = dinv_pad[c * SHARD:(c + 1) * SHARD]
        keep = mkeep[c]
        cols = np.zeros((P, 4 * T), np.float32)
        cols[:, 0 * T:1 * T] = (keep * dloc).reshape(T, P).T
        cols[:, 1 * T:2 * T] = ((1.0 - keep) * dloc).reshape(T, P).T
        cols[:, 2 * T:3 * T] = dloc.reshape(T, P).T
        cols[:, 3 * T:4 * T] = (keep * dloc).reshape(T, P).T
        in_maps.append({
            "x_shard": xs,
            "src_rows": srcs[c],
            "dst_local": dstl[c],
            "dinv_row": np.ascontiguousarray(dloc[None, :]),
            "cols": cols,
            "token_rep": np.ascontiguousarray(
                np.broadcast_to(token[None, :], (P, IN))),
            "iota_row": np.ascontiguousarray(np.tile(
                np.arange(P, dtype=np.float32), B)[None, :].astype(BF)),
            "ident_bf": ident_bf,
            "ones_col": np.ones((P, 1), np.float32),
            "w1": w1,
            "w2a": np.ascontiguousarray(w2[:P]),
            "w2b": np.ascontiguousarray(w2[P:]),
            "wd": wd,
            "gb": gb,
            "a_rep": a_rep,
            "mrow_l": mlocal[c],
            "mval": mvalid[c],
        })
    return nc, in_maps


def kernel(**inputs):
    import os
    nc, in_maps = prepare(inputs)
    res = run_bass_kernel_spmd(nc, in_maps, core_ids=list(range(NC)),
                               trace=bool(os.environ.get("KTRACE")))
    kernel._last_results = res
    loss = res.results[0]["loss"][0, 0]
    return np.float32(loss).reshape(())
